# revision 1
# baseline (speedup 1.0000x reference)
"""BetaGNN message-passing kernel for 8 Trainium2 NeuronCores.

Strategy (dest-row sharding, 6250 nodes/core):
  - Host relabels nodes: sorted by in-degree, dealt round-robin to cores so
    every core's tile t has near-identical max-degree -> uniform chunk counts.
  - Hop 1 (AH = A @ relu(x @ W_in^T + b)): no gather at all. Host pre-gathers
    the 3-wide input features per edge (plus a ones column for the bias);
    the PE recomputes h per edge-slot: one K=4 matmul per 128-edge chunk.
    Edge values are folded into the relu via the activation engine's
    per-partition scale; a constant-identity matmul accumulates chunks into
    the per-tile PSUM (each chunk holds at most one edge per dest column).
  - AH (bf16) is AllGathered across the 8 cores (ncfw collective).
  - Hop 2 (A2H = A @ AH): dma_gather of row PAIRS (1KB elems) from the
    gathered table, so int16 indices only need to reach 25000. The right
    half of each pair is selected by splitting the edge value into an
    even/odd scale vector (the wrong half is scaled by 0).
  - Dense tail in transposed layout (PE transposes AH/A2H tiles):
    h2^T = relu(W1 AH^T + W2 A2H^T), g = softplus(W_out h2^T + b_out).
"""

import sys

for _p in ("/opt/trn_rl_repo", "/root/.axon_site/_ro/trn_rl_repo"):
    if _p not in sys.path:
        sys.path.insert(0, _p)

import numpy as np
import ml_dtypes

import concourse.bacc as bacc
import concourse.bass as bass
import concourse.mybir as mybir
from concourse import tile
from concourse.bass_utils import run_bass_kernel_spmd
from concourse import bass_utils as _bu

# Enable walrus LDWEIGHTS dedup: phase A reloads the same identity/weight
# tiles between matmuls; the default =false flag forces a reload per matmul.
_orig_gwa = _bu.get_walrus_args
def _gwa(*a, **k):
    return [str(x).replace("--enable-ldw-opt=false", "--enable-ldw-opt=true")
            for x in _orig_gwa(*a, **k)]
_bu.get_walrus_args = _gwa

F32 = mybir.dt.float32
F32R = mybir.dt.float32r
BF16 = mybir.dt.bfloat16
I16 = mybir.dt.int16
AF = mybir.ActivationFunctionType

MAX_CALL_CHUNKS = 12      # <=12 chunks (1536 idxs) per dma_gather call
XGRP = 8                  # x4 feature chunks loaded per DMA


class Cfg:
    def __init__(self, P, E, nc=8, hid=256):
        assert P % (nc * 2) == 0
        self.P, self.E, self.NC, self.HID = P, E, nc, hid
        self.NPC = P // nc                    # nodes per core
        self.NT = (self.NPC + 127) // 128     # dest tiles per core
        self.NPAD = self.NT * 128
        self.BLK = []
        off = 0
        while off < self.NPAD:
            w = min(512, self.NPAD - off)
            self.BLK.append((off, w))
            off += w


def _plan(cfg, deg):
    P, NC, NT = cfg.P, cfg.NC, cfg.NT
    order = np.argsort(-deg, kind="stable")
    rank = np.empty(P, np.int64)
    rank[order] = np.arange(P)
    core_of = rank % NC
    local_of = rank // NC
    gid = core_of * cfg.NPC + local_of
    degs_sorted = deg[order]
    NCHUNK = []
    for t in range(NT):
        NCHUNK.append(max(1, int(degs_sorted[t * 128 * NC])))
    NCHUNK = np.array(NCHUNK, np.int64)
    tile_off = np.concatenate([[0], np.cumsum(NCHUNK)])
    calls = []
    for t in range(NT):
        rem, c0 = int(NCHUNK[t]), 0
        while rem:
            g = min(MAX_CALL_CHUNKS, rem)
            calls.append((t, c0, g))
            c0 += g
            rem -= g
    return core_of, local_of, gid, NCHUNK, tile_off, int(tile_off[-1]), calls


def _prepare(cfg, beta, degree, A_rows, A_cols, A_vals,
             W_in, b_in, W_mp1, W_mp2, W_out, b_out):
    P, E, NC, NPC = cfg.P, cfg.E, cfg.NC, cfg.NPC
    deg = np.bincount(A_rows, minlength=P).astype(np.int64)
    core_of, local_of, gid, NCHUNK, tile_off, TC, calls = _plan(cfg, deg)
    NSLOT = TC * 128

    d_gid = gid[A_rows.astype(np.int64)]
    oe = np.argsort(d_gid, kind="stable")
    sd = d_gid[oe]
    first = np.r_[True, sd[1:] != sd[:-1]]
    cumstart = np.maximum.accumulate(np.where(first, np.arange(E), 0))
    chunk = np.arange(E) - cumstart
    e_core = sd // NPC
    e_local = sd % NPC
    e_col = e_local % 128
    e_k = tile_off[e_local // 128] + chunk
    e_slot = e_k * 128 + e_col

    src = A_cols.astype(np.int64)[oe]
    vals = A_vals[oe].astype(np.float32)
    sgid = gid[src]
    pidx = (sgid // 2).astype(np.int16)
    half = (sgid % 2).astype(np.int64)

    x4_all = np.stack([beta[:, 0], beta[:, 0] ** 2, degree[:, 0],
                       np.ones(P, np.float32)], axis=0).astype(np.float32)

    NIDXCOL = NSLOT // 16
    per_core = []
    for c in range(NC):
        m = e_core == c
        sl, km, cm, hm = e_slot[m], e_k[m], e_col[m], half[m]
        x4T = np.zeros((4, NSLOT), np.float32)
        x4T[:, sl] = x4_all[:, src[m]]
        # quad-packed layout: chunk 4q+j -> partitions 32j..32j+4, cols q*128
        NQ = (TC + 3) // 4
        x4c = np.zeros((4, NQ * 4, 128), np.float32)
        x4c[:, :TC, :] = x4T.reshape(4, TC, 128)
        x4q = np.zeros((128, NQ * 128), np.float32)
        for j in range(4):
            x4q[32 * j:32 * j + 4, :] = (
                x4c[:, j::4, :].reshape(4, NQ * 128))
        v1 = np.zeros((128, TC), np.float32)
        v1[cm, km] = vals[m]
        vL = np.zeros((128, TC), np.float32)
        vR = np.zeros((128, TC), np.float32)
        vL[cm[hm == 0], km[hm == 0]] = vals[m][hm == 0]
        vR[cm[hm == 1], km[hm == 1]] = vals[m][hm == 1]
        # diagonal S matrices [128, TC, 128]: sL[p, k, p] = vL[p, k]
        sL = np.zeros((128, TC, 128), ml_dtypes.bfloat16)
        sR = np.zeros((128, TC, 128), ml_dtypes.bfloat16)
        pp = np.arange(128)
        sL[pp, :, pp] = vL.astype(ml_dtypes.bfloat16)
        sR[pp, :, pp] = vR.astype(ml_dtypes.bfloat16)
        sL = sL.reshape(128, TC * 128)
        sR = sR.reshape(128, TC * 128)
        pslot = np.zeros(NSLOT, np.int16)
        pslot[sl] = pidx[m]
        idxh = np.zeros((128, NIDXCOL), np.int16)
        col0 = 0
        soff = 0
        for (t, c0, g) in calls:
            ni = g * 128
            blockv = pslot[soff:soff + ni].reshape(ni // 16, 16).T
            for q in range(8):
                idxh[16 * q:16 * (q + 1), col0:col0 + ni // 16] = blockv
            col0 += ni // 16
            soff += ni
        per_core.append(dict(x4q=x4q, v1=v1, sL=sL, sR=sR, idx=idxh))

    wiT = np.concatenate([W_in.T.astype(np.float32),
                          b_in[None, :].astype(np.float32)], axis=0)
    wiT4 = np.zeros((128, wiT.shape[1]), np.float32)
    for j in range(4):
        wiT4[32 * j:32 * j + 4, :] = wiT
    consts = dict(
        wit=wiT4,
        w1t=np.ascontiguousarray(W_mp1.T.astype(np.float32)),
        w2t=np.ascontiguousarray(W_mp2.T.astype(np.float32)),
        wot=np.ascontiguousarray(W_out.T.astype(np.float32)),
        bout=np.full((128, 1), float(np.asarray(b_out).reshape(-1)[0]),
                     np.float32),
        idn16=np.eye(128, dtype=np.float32).astype(ml_dtypes.bfloat16),
        idn32=np.eye(128, dtype=np.float32),
    )
    meta = dict(NCHUNK=tuple(int(x) for x in NCHUNK), calls=tuple(calls),
                TC=TC, NSLOT=NSLOT, NIDXCOL=NIDXCOL, NQ=(TC + 3) // 4)
    return per_core, consts, meta, (core_of, local_of)


def _build(cfg, meta):
    NT, NPC, NPAD, HID, NC, P = (cfg.NT, cfg.NPC, cfg.NPAD, cfg.HID,
                                 cfg.NC, cfg.P)
    NCHUNK = meta["NCHUNK"]
    calls = meta["calls"]
    TC, NSLOT, NIDXCOL = meta["TC"], meta["NSLOT"], meta["NIDXCOL"]
    tile_off = np.concatenate([[0], np.cumsum(NCHUNK)])
    NBLK = len(cfg.BLK)

    nc = bacc.Bacc("TRN2", target_bir_lowering=False, debug=False)
    NQ = meta["NQ"]
    x4T_d = nc.dram_tensor("x4t", [128, NQ * 128], F32R, kind="ExternalInput")
    v1_d = nc.dram_tensor("v1", [128, TC], F32, kind="ExternalInput")
    sL_d = nc.dram_tensor("sl", [128, TC * 128], BF16, kind="ExternalInput")
    sR_d = nc.dram_tensor("sr", [128, TC * 128], BF16, kind="ExternalInput")
    idx_d = nc.dram_tensor("idx", [128, NIDXCOL], I16, kind="ExternalInput")
    wiT_d = nc.dram_tensor("wit", [128, HID], F32R, kind="ExternalInput")
    w1T_d = nc.dram_tensor("w1t", [HID, HID], F32R, kind="ExternalInput")
    w2T_d = nc.dram_tensor("w2t", [HID, HID], F32R, kind="ExternalInput")
    woT_d = nc.dram_tensor("wot", [HID, 1], F32R, kind="ExternalInput")
    bout_d = nc.dram_tensor("bout", [128, 1], F32, kind="ExternalInput")
    idn16_d = nc.dram_tensor("idn16", [128, 128], BF16, kind="ExternalInput")
    idn32_d = nc.dram_tensor("idn32", [128, 128], F32, kind="ExternalInput")
    g_d = nc.dram_tensor("g", [1, NBLK * 512], F32, kind="ExternalOutput")

    ah_bounce = nc.dram_tensor("ah_bounce", [NPC, HID], BF16)
    ah_full = nc.dram_tensor("ah_full", [P, HID], BF16, addr_space="Shared")

    with tile.TileContext(nc) as tc:
        with (
            tc.tile_pool(name="const", bufs=1) as constp,
            tc.tile_pool(name="xs", bufs=3) as xsp,
            tc.tile_pool(name="msgs", bufs=6) as msgp,
            tc.tile_pool(name="stage", bufs=3) as stagep,
            tc.tile_pool(name="resid", bufs=1) as residp,
            tc.tile_pool(name="pair", bufs=3) as pairp,
            tc.tile_pool(name="ph", bufs=2, space="PSUM") as php,
            tc.tile_pool(name="pz", bufs=2, space="PSUM") as pzp,
            tc.tile_pool(name="pt", bufs=2, space="PSUM") as ptp,
        ):
            wiT = constp.tile([128, HID], F32R, tag="wiT", name="wiT")
            nc.sync.dma_start(wiT[:], wiT_d[:])
            w1T = [constp.tile([128, HID], F32R, tag=f"w1_{k}", name=f"w1_{k}") for k in (0, 1)]
            w2T = [constp.tile([128, HID], F32R, tag=f"w2_{k}", name=f"w2_{k}") for k in (0, 1)]
            for k in (0, 1):
                nc.sync.dma_start(w1T[k][:], w1T_d[128 * k:128 * (k + 1), :])
                nc.sync.dma_start(w2T[k][:], w2T_d[128 * k:128 * (k + 1), :])
            woT = constp.tile([128, 2], F32R, tag="woT", name="woT")
            nc.sync.dma_start(woT[:, 0:1], woT_d[0:128, :])
            nc.sync.dma_start(woT[:, 1:2], woT_d[128:256, :])
            bout = constp.tile([128, 1], F32, tag="bout", name="bout")
            nc.sync.dma_start(bout[:], bout_d[:])
            idn16 = constp.tile([128, 128], BF16, tag="idn16", name="idn16")
            nc.sync.dma_start(idn16[:], idn16_d[:])
            idn32 = constp.tile([128, 128], F32, tag="idn32", name="idn32")
            nc.sync.dma_start(idn32[:], idn32_d[:])
            v1 = constp.tile([128, TC], F32, tag="v1", name="v1")
            nc.sync.dma_start(v1[:], v1_d[:])
            idx = constp.tile([128, NIDXCOL], I16, tag="idx", name="idx")
            nc.sync.dma_start(idx[:], idx_d[:])

            ahT = [residp.tile([128, NPAD], F32R, tag=f"ahT{m}", name=f"ahT{m}")
                   for m in (0, 1)]
            a2T = [residp.tile([128, NPAD], F32R, tag=f"a2T{m}", name=f"a2T{m}")
                   for m in (0, 1)]

            # ---- phase A: hop 1 (quad-packed K=4 matmuls, 8-groups) ----
            t = 0
            pz = None
            TCn = int(tile_off[-1])
            tileends = []
            for g8 in range(0, TCn, 8):
                khi = min(g8 + 8, TCn)
                xs = xsp.tile([128, 2 * 128], F32R, tag="xs", name="xs")
                q0 = g8 // 4
                hi = min((q0 + 2) * 128, NQ * 128)
                nc.sync.dma_start(xs[:, :hi - q0 * 128],
                                  x4T_d[:, q0 * 128:hi])
                # 8 h-matmuls, one PSUM bank each
                phs = []
                for k in range(g8, khi):
                    j, half = k % 4, (k - g8) // 4
                    ph = php.tile([128, 512], F32, tag="ph", name="ph",
                                  bufs=4)
                    nc.tensor.matmul(
                        ph[:, :HID],
                        lhsT=xs[32 * j:32 * j + 4,
                                half * 128:(half + 1) * 128],
                        rhs=wiT[32 * j:32 * j + 4, :],
                        start=True, stop=True, skip_group_check=True,
                        tile_position=(32 * j, 0))
                    phs.append(ph)
                # 8 relus
                ms = []
                for k in range(g8, khi):
                    m = msgp.tile([128, HID], BF16, tag="m1", name="m1",
                                  bufs=10)
                    if k % 2 == 0:
                        nc.scalar.activation(m[:], phs[k - g8][:, :HID],
                                             AF.Relu, scale=v1[:, k:k + 1])
                    else:
                        nc.vector.tensor_scalar(
                            m[:], phs[k - g8][:, :HID], v1[:, k:k + 1], 0.0,
                            op0=mybir.AluOpType.mult,
                            op1=mybir.AluOpType.max)
                    ms.append(m)
                # 8 accumulate matmuls (tile boundaries handled per k)
                for k in range(g8, khi):
                    if k == int(tile_off[t]):
                        pz = pzp.tile([128, 512], F32, tag="acc", name="acc")
                    nc.tensor.matmul(
                        pz[:, :HID], lhsT=idn16[:], rhs=ms[k - g8][:],
                        start=(k == int(tile_off[t])),
                        stop=(k == int(tile_off[t + 1]) - 1),
                        skip_group_check=True)
                    if k == int(tile_off[t + 1]) - 1:
                        tileends.append((t, pz))
                        t += 1
                # emit epilogues for any tiles completed in this group
                for (tt, pzv) in tileends:
                    ah = stagep.tile([128, HID], F32, tag="ah", name="ah")
                    nc.vector.tensor_copy(ah[:], pzv[:, :HID])
                    ahb = stagep.tile([128, HID], BF16, tag="ahb",
                                      name="ahb")
                    nc.scalar.activation(ahb[:], pzv[:, :HID], AF.Copy)
                    rows = min(128, NPC - tt * 128)
                    nc.sync.dma_start(ah_bounce[tt * 128:tt * 128 + rows, :],
                                      ahb[:rows, :])
                    for mh in (0, 1):
                        pt = ptp.tile([128, 512], F32, tag="pt", name="pt")
                        nc.tensor.transpose(
                            pt[:, :128], ah[:, mh * 128:(mh + 1) * 128],
                            idn32[:])
                        nc.vector.tensor_copy(
                            ahT[mh][:, tt * 128:(tt + 1) * 128], pt[:, :128])
                tileends = []

            # ---- phase B: allgather ----
            nc.gpsimd.collective_compute(
                "AllGather", mybir.AluOpType.bypass,
                replica_groups=[list(range(NC))],
                ins=[ah_bounce.ap().opt()],
                outs=[ah_full.ap().opt()],
            )
            ah_pairs = ah_full.ap().rearrange("(a b) c -> a (b c)", b=2)

            # ---- phase C: hop 2 ----
            ci = 0
            col0 = 0
            for t in range(NT):
                nch = NCHUNK[t]
                k0 = int(tile_off[t])
                pz = pzp.tile([128, 512], F32, tag="acc", name="acc")
                first = True
                done = 0
                while done < nch:
                    (tt, c0, g) = calls[ci]
                    ni = g * 128
                    pr = pairp.tile([128, MAX_CALL_CHUNKS, 2 * HID], BF16,
                                    tag="pair", name="pair")
                    nc.gpsimd.dma_gather(
                        pr[:, :g, :], ah_pairs,
                        idx[:, col0:col0 + ni // 16],
                        ni, ni, 2 * HID, single_packet=False)
                    kb = (k0 + done) * 128
                    sdl = msgp.tile([128, MAX_CALL_CHUNKS * 128], BF16,
                                    tag="sdl", name="sdl", bufs=2)
                    nc.sync.dma_start(sdl[:, :ni], sL_d[:, kb:kb + ni])
                    sdr = msgp.tile([128, MAX_CALL_CHUNKS * 128], BF16,
                                    tag="sdr", name="sdr", bufs=2)
                    nc.sync.dma_start(sdr[:, :ni], sR_d[:, kb:kb + ni])
                    for cc in range(g):
                        nc.tensor.matmul(
                            pz[:, :HID],
                            lhsT=sdl[:, cc * 128:(cc + 1) * 128],
                            rhs=pr[:, cc, 0:HID],
                            start=first, stop=False, skip_group_check=True)
                        first = False
                        nc.tensor.matmul(
                            pz[:, :HID],
                            lhsT=sdr[:, cc * 128:(cc + 1) * 128],
                            rhs=pr[:, cc, HID:2 * HID],
                            start=False, stop=(done + cc == nch - 1),
                            skip_group_check=True)
                    done += g
                    col0 += ni // 16
                    ci += 1
                a2 = stagep.tile([128, HID], F32, tag="a2", name="a2")
                nc.vector.tensor_copy(a2[:], pz[:, :HID])
                for mh in (0, 1):
                    pt = ptp.tile([128, 512], F32, tag="pt", name="pt")
                    nc.tensor.transpose(
                        pt[:, :128], a2[:, mh * 128:(mh + 1) * 128], idn32[:])
                    nc.vector.tensor_copy(
                        a2T[mh][:, t * 128:(t + 1) * 128], pt[:, :128])

            # ---- phase D: dense tail ----
            for b, (off, w) in enumerate(cfg.BLK):
                h2 = []
                for mh in (0, 1):
                    pd = pzp.tile([128, 512], F32, tag="acc", name="acc")
                    n = 0
                    for (wt, xt) in ((w1T, ahT), (w2T, a2T)):
                        for k in (0, 1):
                            nc.tensor.matmul(
                                pd[:, :w],
                                lhsT=wt[k][:, mh * 128:(mh + 1) * 128]
                                ,
                                rhs=xt[k][:, off:off + w],
                                start=(n == 0), stop=(n == 3),
                                skip_group_check=True)
                            n += 1
                    ht = stagep.tile([128, 512], F32R, tag="h2t", name="h2t")
                    nc.scalar.activation(ht[:, :w], pd[:, :w], AF.Relu)
                    h2.append(ht)
                pg = ptp.tile([1, 512], F32, tag="pt", name="pt")
                for k in (0, 1):
                    nc.tensor.matmul(pg[:, :w],
                                     lhsT=woT[:, k:k + 1],
                                     rhs=h2[k][:, :w],
                                     start=(k == 0), stop=(k == 1),
                                     skip_group_check=True)
                gb = stagep.tile([1, 512], F32, tag="gbuf", name="gb",
                                 bufs=4)
                nc.vector.tensor_copy(gb[0:1, :w], pg[:, :w])
                ge = stagep.tile([1, 512], F32, tag="gbuf", name="ge",
                                 bufs=4)
                nc.scalar.activation(ge[0:1, :w], gb[0:1, :w], AF.Exp,
                                     bias=bout[0:1, :])
                go = stagep.tile([1, 512], F32, tag="gbuf", name="go",
                                 bufs=4)
                nc.scalar.activation(go[0:1, :w], ge[0:1, :w], AF.Ln,
                                     bias=1.0)
                nc.sync.dma_start(g_d[0:1, off:off + w], go[0:1, :w])



    nc.compile()
    return nc


_COMPILED = {}


def _get_compiled(cfg, meta):
    key = (cfg.P, cfg.E, meta["NCHUNK"], meta["calls"])
    if key not in _COMPILED:
        _COMPILED[key] = _build(cfg, meta)
    return _COMPILED[key]


def run(cfg, inputs, trace=False):
    per_core, consts, meta, (core_of, local_of) = _prepare(cfg, **inputs)
    ncobj = _get_compiled(cfg, meta)
    in_maps = []
    for c in range(cfg.NC):
        pc = per_core[c]
        im = {"x4t": pc["x4q"], "v1": pc["v1"], "sl": pc["sL"],
              "sr": pc["sR"], "idx": pc["idx"]}
        im.update({k: np.asarray(v) for k, v in consts.items()})
        in_maps.append(im)
    res = run_bass_kernel_spmd(ncobj, in_maps, list(range(cfg.NC)),
                               trace=trace)
    g = np.empty(cfg.P, np.float32)
    for c in range(cfg.NC):
        go = np.asarray(res.results[c]["g"]).reshape(-1)
        mine = core_of == c
        g[mine] = go[local_of[mine]]
    return g.reshape(cfg.P, 1), res


def kernel(**inputs):
    cfg = Cfg(P=50000, E=800000)
    g, _ = run(cfg, inputs)
    return g



# revision 11
# speedup vs baseline: 1.7191x; 1.7191x over previous
"""BetaGNN message-passing kernel for 8 Trainium2 NeuronCores.

Strategy (dest-row sharding, 6250 nodes/core):
  - Host relabels nodes: sorted by in-degree, dealt round-robin to cores so
    every core's tile t has near-identical max-degree -> uniform chunk counts.
  - Hop 1 (AH = A @ relu(x @ W_in^T + b)): no gather. Host pre-gathers the
    3-wide input features per edge (plus a ones column for the bias); the PE
    recomputes h per edge-slot (one K=4 matmul per 128-edge chunk). Edge
    values (x16) fold into the relu via per-partition scale; messages are
    written fp8 and accumulated two chunks at a time with a DoubleRow
    identity matmul (fp8 perf mode, 0.5 cyc/row).
  - The local AH rows (x16, fp8) are AllGathered in TWO halves: tiles 0..31
    right after they finish (overlapping the rest of hop 1), tiles 32..48
    after. Each half lands in a compact per-half table so hop-2 gather
    indices stay int16 (max 32767).
  - Hop 2 (A2H = A @ AH): per dest tile, edges are packed 128/chunk with a
    general scatter matrix S (fp8, values x16) routing slot -> dest row, so
    chunks need no max-degree padding. Rows are dma_gathered (256B fp8) on 4
    SWDGE queues; pairs of chunks accumulate with one DoubleRow matmul.
  - Dense tail in transposed layout: AH/A2H tiles are transposed via fp8
    matmuls against scaled identities into [128, 2, NPAD] fp8 residents;
    h2^T = relu(W1 AH^T + W2 A2H^T) and g = softplus(W_out h2^T + b_out)
    run one 512-col block at a time, interleaved into hop 2 (PE work under
    the gather DMA). All fp8 scale factors are powers of two (exact).
"""

import sys

for _p in ("/opt/trn_rl_repo", "/root/.axon_site/_ro/trn_rl_repo"):
    if _p not in sys.path:
        sys.path.insert(0, _p)

import numpy as np
import ml_dtypes

import concourse.bacc as bacc
import concourse.bass as bass
import concourse.mybir as mybir
from concourse import tile
from concourse.bass_utils import run_bass_kernel_spmd
from concourse import bass_utils as _bu

F32 = mybir.dt.float32
F32R = mybir.dt.float32r
BF16 = mybir.dt.bfloat16
FP8 = mybir.dt.float8e4
I16 = mybir.dt.int16
AF = mybir.ActivationFunctionType
DR = mybir.MatmulPerfMode.DoubleRow
NPFP8 = ml_dtypes.float8_e4m3fn

MAX_CALL_CHUNKS = 12      # <=12 chunks (1536 idxs) per dma_gather call
NQUEUES = 4               # SWDGE queues for gather concurrency


class Cfg:
    def __init__(self, P, E, nc=8, hid=256):
        assert P % (nc * 2) == 0
        self.P, self.E, self.NC, self.HID = P, E, nc, hid
        self.NPC = P // nc                    # nodes per core
        self.NT = (self.NPC + 127) // 128     # dest tiles per core
        self.NPAD = self.NT * 128
        self.SPLIT_T = 32 if self.NT > 40 else max(1, 2 * self.NT // 3)
        self.S1 = self.SPLIT_T * 128          # locals in collective half 1
        self.S2 = self.NPC - self.S1
        self.BLK = []
        off = 0
        while off < self.NPAD:
            w = min(512, self.NPAD - off)
            self.BLK.append((off, w))
            off += w


def _plan(cfg, deg):
    """Hop-1 plan: chunk count per tile = max in-degree in the tile."""
    P, NC, NT = cfg.P, cfg.NC, cfg.NT
    order = np.argsort(-deg, kind="stable")
    rank = np.empty(P, np.int64)
    rank[order] = np.arange(P)
    core_of = rank % NC
    local_of = rank // NC
    gid = core_of * cfg.NPC + local_of
    degs_sorted = deg[order]
    NCHUNK = []
    for t in range(NT):
        NCHUNK.append(max(1, int(degs_sorted[min(t * 128 * NC, P - 1)])))
    NCHUNK = np.array(NCHUNK, np.int64)
    tile_off = np.concatenate([[0], np.cumsum(NCHUNK)])
    return core_of, local_of, gid, NCHUNK, tile_off, int(tile_off[-1])


def _split_calls(nchunks):
    """Split a chunk count into gather calls <= MAX_CALL_CHUNKS, keeping
    every non-final call even so DoubleRow pairs never straddle calls."""
    out = []
    rem = nchunks
    while rem:
        g = min(MAX_CALL_CHUNKS, rem)
        if g < rem and g % 2:
            g -= 1
        out.append(g)
        rem -= g
    return out


def _prepare(cfg, beta, degree, A_rows, A_cols, A_vals,
             W_in, b_in, W_mp1, W_mp2, W_out, b_out):
    P, E, NC, NPC, NT = cfg.P, cfg.E, cfg.NC, cfg.NPC, cfg.NT
    S1, S2 = cfg.S1, cfg.S2
    deg = np.bincount(A_rows, minlength=P).astype(np.int64)
    core_of, local_of, gid, NCHUNK, tile_off, TC = _plan(cfg, deg)

    # ---- hop-1 edge slots (slot column == dest column, like v1) ----
    d_gid = gid[A_rows.astype(np.int64)]
    oe = np.argsort(d_gid, kind="stable")
    sd = d_gid[oe]
    first = np.r_[True, sd[1:] != sd[:-1]]
    cumstart = np.maximum.accumulate(np.where(first, np.arange(E), 0))
    chunk = np.arange(E) - cumstart
    e_core = sd // NPC
    e_local = sd % NPC
    e_col = e_local % 128
    e_k = tile_off[e_local // 128] + chunk
    e_slot = e_k * 128 + e_col
    src1 = A_cols.astype(np.int64)[oe]
    vals1 = A_vals[oe].astype(np.float32)

    x4_all = np.stack([beta[:, 0], beta[:, 0] ** 2, degree[:, 0],
                       np.ones(P, np.float32)], axis=0).astype(np.float32)

    # ---- hop-2 edge plan: sort by (core, tile, bucket) ----
    s_gid = gid[A_cols.astype(np.int64)]
    c2_core = d_gid // NPC
    c2_loc = d_gid % NPC
    c2_tile = c2_loc // 128
    c2_dcol = c2_loc % 128
    s_core = s_gid // NPC
    s_loc = s_gid % NPC
    c2_b = (s_loc >= S1).astype(np.int64)
    c2_tidx = np.where(c2_b == 0, s_core * S1 + s_loc,
                       s_core * S2 + (s_loc - S1)).astype(np.int64)
    o2 = np.lexsort((c2_b, c2_tile, c2_core))
    g_core = c2_core[o2]
    g_tile = c2_tile[o2]
    g_b = c2_b[o2]
    g_dcol = c2_dcol[o2]
    g_tidx = c2_tidx[o2]
    g_val = A_vals[o2].astype(np.float32)
    key = (g_core * NT + g_tile) * 2 + g_b
    kfirst = np.r_[True, key[1:] != key[:-1]]
    kcum = np.maximum.accumulate(np.where(kfirst, np.arange(E), 0))
    g_pos = np.arange(E) - kcum        # position within (core,tile,bucket)

    # shared SPMD structure: chunk counts per (tile,bucket) = max over cores
    # (cores with fewer edges pad with zero-S / index-0 chunks)
    cnt_all = np.zeros((NC, NT, 2), np.int64)
    np.add.at(cnt_all, (g_core, g_tile, g_b), 1)
    nch = -(-cnt_all.max(axis=0) // 128)          # [NT, 2] ceil
    nch[cnt_all.sum(axis=0) == 0] = 0
    flat = nch.reshape(-1)
    cbase = np.concatenate([[0], np.cumsum(flat)]).astype(np.int64)
    TOT = int(cbase[-1])
    calls = []          # (tile, bucket, chunk_base, g)
    for t in range(NT):
        for b in (0, 1):
            base = int(cbase[t * 2 + b])
            for g in _split_calls(int(nch[t, b])):
                calls.append((t, b, base, g))
                base += g
    NIC = sum(g * 128 // 16 for (_, _, _, g) in calls)

    per_core = []
    for c in range(NC):
        # ---- hop 1 arrays ----
        m1 = e_core == c
        sl1 = e_slot[m1]
        NSLOT = TC * 128
        x4T = np.zeros((4, NSLOT), np.float32)
        x4T[:, sl1] = x4_all[:, src1[m1]]
        v1 = np.zeros((128, TC), np.float32)
        v1[e_col[m1], e_k[m1]] = 16.0 * vals1[m1]

        # quad-pack features per part (part1 chunks [0,c1), part2 [c1,TC))
        c1 = int(tile_off[cfg.SPLIT_T])
        xparts = []
        for lo, hi in ((0, c1), (c1, TC)):
            n = hi - lo
            NQ = (n + 3) // 4
            x4c = np.zeros((4, NQ * 4, 128), np.float32)
            x4c[:, :n, :] = x4T[:, lo * 128:hi * 128].reshape(4, n, 128)
            x4q = np.zeros((128, NQ * 128), np.float32)
            for j in range(4):
                x4q[32 * j:32 * j + 4, :] = (
                    x4c[:, j::4, :].reshape(4, NQ * 128))
            xparts.append(x4q)

        # ---- hop 2 arrays ----
        m2 = g_core == c
        e_key = (g_tile[m2] * 2 + g_b[m2])
        e_chunk = cbase[e_key] + g_pos[m2] // 128
        e_p = g_pos[m2] % 128
        S8 = np.zeros((128, TOT * 128), np.float32)
        S8[e_p, e_chunk * 128 + g_dcol[m2]] = 16.0 * g_val[m2]
        S8 = S8.astype(NPFP8)
        slot_idx = np.zeros(TOT * 128, np.int64)
        slot_idx[e_chunk * 128 + e_p] = g_tidx[m2]

        # idx stream (wrapped in 16 partitions, replicated x8)
        idxh = np.zeros((128, NIC), np.int16)
        col0 = 0
        for (t, b, base, g) in calls:
            ni = g * 128
            blockv = slot_idx[base * 128:base * 128 + ni].astype(np.int16)
            blockv = blockv.reshape(ni // 16, 16).T
            for q in range(8):
                idxh[16 * q:16 * (q + 1), col0:col0 + ni // 16] = blockv
            col0 += ni // 16
        per_core.append(dict(x4a=xparts[0], x4b=xparts[1], v1=v1,
                             s8=S8, idx=idxh))

    # ---- constants (power-of-two scaled for fp8) ----
    wiT = np.concatenate([W_in.T.astype(np.float32),
                          b_in[None, :].astype(np.float32)], axis=0)
    wiT4 = np.zeros((128, wiT.shape[1]), np.float32)
    for j in range(4):
        wiT4[32 * j:32 * j + 4, :] = wiT

    def pack_w(W, scale):
        # [p, i, m] = scale * W[m, i*128 + p]
        w = (scale * W.T.astype(np.float32)).reshape(2, 128, W.shape[0])
        return np.ascontiguousarray(
            w.transpose(1, 0, 2)).astype(NPFP8)

    idn2 = np.zeros((128, 2, 128), np.float32)
    idn2[np.arange(128), 0, np.arange(128)] = 1.0
    idn2[np.arange(128), 1, np.arange(128)] = 1.0
    sidn8 = (np.eye(128, dtype=np.float32) * 0.125).astype(NPFP8)
    sidn4 = (np.eye(128, dtype=np.float32) * 0.25).astype(NPFP8)

    consts = dict(
        wit=wiT4,
        w1s=pack_w(W_mp1, 32.0),
        w2s=pack_w(W_mp2, 16.0),
        wos=(16.0 * W_out.reshape(2, 128).T.reshape(128, 2, 1)
             .astype(np.float32)).astype(NPFP8),
        bout=np.full((128, 1), float(np.asarray(b_out).reshape(-1)[0]),
                     np.float32),
        idn2=idn2.astype(NPFP8),
        sidn8=sidn8,
        sidn4=sidn4,
    )
    meta = dict(NCHUNK=tuple(int(x) for x in NCHUNK), TC=TC,
                nch=tuple(int(x) for r in nch for x in r),
                calls=tuple(calls), TOT=TOT, NIC=NIC,
                NQ1=(int(tile_off[cfg.SPLIT_T]) + 3) // 4,
                NQ2=(TC - int(tile_off[cfg.SPLIT_T]) + 3) // 4)
    return per_core, consts, meta, (core_of, local_of)


def _build(cfg, meta):
    NT, NPC, NPAD, HID, NC, P = (cfg.NT, cfg.NPC, cfg.NPAD, cfg.HID,
                                 cfg.NC, cfg.P)
    S1, S2, SPLIT_T = cfg.S1, cfg.S2, cfg.SPLIT_T
    NCHUNK = meta["NCHUNK"]
    TC, NIC, TOT = meta["TC"], meta["NIC"], meta["TOT"]
    calls = meta["calls"]
    nch = np.array(meta["nch"], np.int64).reshape(NT, 2)
    tile_off = np.concatenate([[0], np.cumsum(NCHUNK)])
    NBLK = len(cfg.BLK)
    NQ1, NQ2 = meta["NQ1"], meta["NQ2"]
    c1 = int(tile_off[SPLIT_T])

    nc = bacc.Bacc("TRN2", target_bir_lowering=False, debug=False,
                   num_swdge_queues=NQUEUES)
    x4a_d = nc.dram_tensor("x4a", [128, NQ1 * 128], F32R, kind="ExternalInput")
    x4b_d = nc.dram_tensor("x4b", [128, NQ2 * 128], F32R, kind="ExternalInput")
    v1_d = nc.dram_tensor("v1", [128, TC], F32, kind="ExternalInput")
    s8_d = nc.dram_tensor("s8", [128, TOT * 128], FP8, kind="ExternalInput")
    idx_d = nc.dram_tensor("idx", [128, NIC], I16, kind="ExternalInput")
    wiT_d = nc.dram_tensor("wit", [128, HID], F32R, kind="ExternalInput")
    w1s_d = nc.dram_tensor("w1s", [128, 2 * HID], FP8, kind="ExternalInput")
    w2s_d = nc.dram_tensor("w2s", [128, 2 * HID], FP8, kind="ExternalInput")
    wos_d = nc.dram_tensor("wos", [128, 2], FP8, kind="ExternalInput")
    bout_d = nc.dram_tensor("bout", [128, 1], F32, kind="ExternalInput")
    idn2_d = nc.dram_tensor("idn2", [128, 2 * 128], FP8, kind="ExternalInput")
    sidn8_d = nc.dram_tensor("sidn8", [128, 128], FP8, kind="ExternalInput")
    sidn4_d = nc.dram_tensor("sidn4", [128, 128], FP8, kind="ExternalInput")
    g_d = nc.dram_tensor("g", [1, NBLK * 512], F32, kind="ExternalOutput")

    bounce1 = nc.dram_tensor("bounce1", [S1, HID], FP8)
    bounce2 = nc.dram_tensor("bounce2", [S2, HID], FP8)
    table1 = nc.dram_tensor("table1", [NC * S1, HID], FP8, addr_space="Shared")
    table2 = nc.dram_tensor("table2", [NC * S2, HID], FP8, addr_space="Shared")

    with tile.TileContext(nc) as tc:
        with (
            tc.tile_pool(name="const", bufs=1) as constp,
            tc.tile_pool(name="xs", bufs=3) as xsp,
            tc.tile_pool(name="msgs", bufs=8) as msgp,
            tc.tile_pool(name="sd", bufs=4) as sdp,
            tc.tile_pool(name="stage", bufs=3) as stagep,
            tc.tile_pool(name="resid", bufs=1) as residp,
            tc.tile_pool(name="pair", bufs=6) as pairp,
            tc.tile_pool(name="ph", bufs=2, space="PSUM") as php,
            tc.tile_pool(name="pz", bufs=2, space="PSUM") as pzp,
            tc.tile_pool(name="pt", bufs=2, space="PSUM") as ptp,
        ):
            wiT = constp.tile([128, HID], F32R, tag="wiT", name="wiT")
            nc.sync.dma_start(wiT[:], wiT_d[:])
            w1s = constp.tile([128, 2, HID], FP8, tag="w1s", name="w1s")
            nc.sync.dma_start(w1s[:], w1s_d[:])
            w2s = constp.tile([128, 2, HID], FP8, tag="w2s", name="w2s")
            nc.sync.dma_start(w2s[:], w2s_d[:])
            wos = constp.tile([128, 2, 1], FP8, tag="wos", name="wos")
            nc.sync.dma_start(wos[:], wos_d[:])
            bout = constp.tile([128, 1], F32, tag="bout", name="bout")
            nc.sync.dma_start(bout[:], bout_d[:])
            idn2 = constp.tile([128, 2, 128], FP8, tag="idn2", name="idn2")
            nc.sync.dma_start(idn2[:], idn2_d[:])
            sidn8 = constp.tile([128, 128], FP8, tag="sidn8", name="sidn8")
            nc.sync.dma_start(sidn8[:], sidn8_d[:])
            sidn4 = constp.tile([128, 128], FP8, tag="sidn4", name="sidn4")
            nc.sync.dma_start(sidn4[:], sidn4_d[:])
            v1 = constp.tile([128, TC], F32, tag="v1", name="v1")
            nc.sync.dma_start(v1[:], v1_d[:])
            idx = constp.tile([128, NIC], I16, tag="idx", name="idx")
            nc.sync.dma_start(idx[:], idx_d[:])

            ahT = residp.tile([128, 2, NPAD], FP8, tag="ahT", name="ahT")
            a2T = residp.tile([128, 2, NPAD], FP8, tag="a2T", name="a2T")

            # ---- phase A: hop 1 ------------------------------------------
            def epilogue_a(t, pz):
                ahb = stagep.tile([128, HID], FP8, tag="ahb", name="ahb")
                nc.scalar.activation(ahb[:], pz[:, :HID], AF.Copy)
                rows = min(128, NPC - t * 128)
                if t < SPLIT_T:
                    nc.sync.dma_start(
                        bounce1[t * 128:t * 128 + rows, :], ahb[:rows, :])
                else:
                    r0 = t * 128 - S1
                    nc.sync.dma_start(
                        bounce2[r0:r0 + rows, :], ahb[:rows, :])
                for mh in (0, 1):
                    pt = ptp.tile([128, 512], F32, tag="pt", name="pt")
                    nc.tensor.matmul(
                        pt[:, :128], lhsT=ahb[:, mh * 128:(mh + 1) * 128],
                        rhs=sidn8[:], start=True, stop=True,
                        skip_group_check=True)
                    nc.vector.tensor_copy(
                        ahT[:, mh, t * 128:(t + 1) * 128], pt[:, :128])

            def phase_a(lo, hi, x4_d, NQp):
                t = int(np.searchsorted(tile_off, lo, side="right")) - 1
                pz = None
                pend = None
                for g8 in range(lo, hi, 8):
                    khi = min(g8 + 8, hi)
                    xs = xsp.tile([128, 2 * 128], F32R, tag="xs", name="xs")
                    q0 = (g8 - lo) // 4
                    chi = min((q0 + 2) * 128, NQp * 128)
                    nc.sync.dma_start(xs[:, :chi - q0 * 128],
                                      x4_d[:, q0 * 128:chi])
                    phs = []
                    for k in range(g8, khi):
                        j, half = (k - lo) % 4, (k - g8) // 4
                        ph = php.tile([128, 512], F32, tag="ph", name="ph",
                                      bufs=4)
                        nc.tensor.matmul(
                            ph[:, :HID],
                            lhsT=xs[32 * j:32 * j + 4,
                                    half * 128:(half + 1) * 128],
                            rhs=wiT[32 * j:32 * j + 4, :],
                            start=True, stop=True, skip_group_check=True,
                            tile_position=(32 * j, 0))
                        phs.append(ph)
                    for k in range(g8, khi):
                        if k == int(tile_off[t]):
                            pz = pzp.tile([128, 512], F32, tag="acc",
                                          name="acc")
                        tstart = (k == int(tile_off[t]))
                        tlast = (k == int(tile_off[t + 1]) - 1)
                        if pend is None:
                            mp = msgp.tile([128, 2, HID], FP8, tag="m1",
                                           name="m1")
                            dst = mp[:, 0, :]
                        else:
                            mp, _ = pend
                            dst = mp[:, 1, :]
                        if k % 2 == 0:
                            nc.scalar.activation(dst, phs[k - g8][:, :HID],
                                                 AF.Relu,
                                                 scale=v1[:, k:k + 1])
                        else:
                            nc.vector.tensor_scalar(
                                dst, phs[k - g8][:, :HID], v1[:, k:k + 1],
                                0.0, op0=mybir.AluOpType.mult,
                                op1=mybir.AluOpType.max)
                        if pend is None and not tlast:
                            pend = (mp, tstart)
                        elif pend is None and tlast:
                            # single leftover chunk closes the tile
                            nc.tensor.matmul(
                                pz[:, :HID], lhsT=idn2[:, 0, :],
                                rhs=mp[:, 0, :], start=tstart, stop=True,
                                skip_group_check=True)
                            epilogue_a(t, pz)
                            t += 1
                        else:
                            mp, pstart = pend
                            nc.tensor.matmul(
                                pz[:, :HID], lhsT=idn2[:], rhs=mp[:],
                                perf_mode=DR, start=pstart, stop=tlast,
                                skip_group_check=True)
                            pend = None
                            if tlast:
                                epilogue_a(t, pz)
                                t += 1

            phase_a(0, c1, x4a_d, NQ1)
            nc.gpsimd.collective_compute(
                "AllGather", mybir.AluOpType.bypass,
                replica_groups=[list(range(NC))],
                ins=[bounce1.ap().opt()],
                outs=[table1.ap().opt()],
            )
            phase_a(c1, TC, x4b_d, NQ2)
            nc.gpsimd.collective_compute(
                "AllGather", mybir.AluOpType.bypass,
                replica_groups=[list(range(NC))],
                ins=[bounce2.ap().opt()],
                outs=[table2.ap().opt()],
            )

            # ---- phase C: hop 2, with the dense tail interleaved ---------
            def dense_block(bidx):
                off, w = cfg.BLK[bidx]
                ht = stagep.tile([128, 2, 512], FP8, tag="h2t", name="h2t")
                for mh in (0, 1):
                    pd = pzp.tile([128, 512], F32, tag="acc", name="acc")
                    nc.tensor.matmul(
                        pd[:, :w], lhsT=w1s[:, :, mh * 128:(mh + 1) * 128],
                        rhs=ahT[:, :, off:off + w], perf_mode=DR,
                        start=True, stop=False, skip_group_check=True)
                    nc.tensor.matmul(
                        pd[:, :w], lhsT=w2s[:, :, mh * 128:(mh + 1) * 128],
                        rhs=a2T[:, :, off:off + w], perf_mode=DR,
                        start=False, stop=True, skip_group_check=True)
                    nc.scalar.activation(ht[:, mh, :w], pd[:, :w], AF.Relu,
                                         scale=0.015625)
                pg = ptp.tile([1, 512], F32, tag="pt", name="pg")
                for i in (0, 1):
                    nc.tensor.matmul(pg[:, :w], lhsT=wos[:, i, :],
                                     rhs=ht[:, i, :w],
                                     start=(i == 0), stop=(i == 1),
                                     skip_group_check=True)
                gb = stagep.tile([1, 512], F32, tag="gbuf", name="gb",
                                 bufs=4)
                nc.vector.tensor_copy(gb[0:1, :w], pg[:, :w])
                ge = stagep.tile([1, 512], F32, tag="gbuf", name="ge",
                                 bufs=4)
                nc.scalar.activation(ge[0:1, :w], gb[0:1, :w], AF.Exp,
                                     bias=bout[0:1, :], scale=0.0625)
                go = stagep.tile([1, 512], F32, tag="gbuf", name="go",
                                 bufs=4)
                nc.scalar.activation(go[0:1, :w], ge[0:1, :w], AF.Ln,
                                     bias=1.0)
                nc.sync.dma_start(g_d[0:1, off:off + w], go[0:1, :w])

            ci = 0
            col0 = 0
            qrr = 0
            for t in range(NT):
                ncht = int(nch[t, 0] + nch[t, 1])
                pz = pzp.tile([128, 512], F32, tag="acc", name="acc")
                done = 0
                while done < ncht:
                    (tt, b, base, g) = calls[ci]
                    assert tt == t
                    ni = g * 128
                    tab = table1 if b == 0 else table2
                    pr = pairp.tile([128, MAX_CALL_CHUNKS, HID], FP8,
                                    tag="pair", name="pair")
                    nc.gpsimd.dma_gather(
                        pr[:, :g, :], tab.ap(),
                        idx[:, col0:col0 + ni // 16],
                        ni, ni, HID, single_packet=False,
                        queue_num=qrr)
                    qrr = (qrr + 1) % NQUEUES
                    sd = sdp.tile([128, MAX_CALL_CHUNKS, 128], FP8,
                                  tag="sdl", name="sdl")
                    nc.sync.dma_start(sd[:, :g, :],
                                      s8_d[:, base * 128:(base + g) * 128])
                    for cc in range(0, g - 1, 2):
                        nc.tensor.matmul(
                            pz[:, :HID], lhsT=sd[:, cc:cc + 2, :],
                            rhs=pr[:, cc:cc + 2, :], perf_mode=DR,
                            start=(done + cc == 0),
                            stop=(done + cc + 2 == ncht),
                            skip_group_check=True)
                    if g % 2:
                        nc.tensor.matmul(
                            pz[:, :HID], lhsT=sd[:, g - 1, :],
                            rhs=pr[:, g - 1, :],
                            start=(done + g - 1 == 0),
                            stop=(done + g == ncht),
                            skip_group_check=True)
                    done += g
                    col0 += ni // 16
                    ci += 1
                a2b = stagep.tile([128, HID], FP8, tag="a2b", name="a2b")
                nc.scalar.activation(a2b[:], pz[:, :HID], AF.Copy,
                                     scale=0.0625)
                for mh in (0, 1):
                    pt = ptp.tile([128, 512], F32, tag="pt", name="pt")
                    nc.tensor.matmul(
                        pt[:, :128], lhsT=a2b[:, mh * 128:(mh + 1) * 128],
                        rhs=sidn4[:], start=True, stop=True,
                        skip_group_check=True)
                    nc.vector.tensor_copy(
                        a2T[:, mh, t * 128:(t + 1) * 128], pt[:, :128])
                if t % 4 == 3:
                    dense_block(t // 4)
            for bidx in range(NT // 4, NBLK):
                dense_block(bidx)

    nc.compile()
    return nc


_COMPILED = {}


def _get_compiled(cfg, meta):
    key = (cfg.P, cfg.E, meta["NCHUNK"], meta["nch"], meta["calls"])
    if key not in _COMPILED:
        _COMPILED[key] = _build(cfg, meta)
    return _COMPILED[key]


def run(cfg, inputs, trace=False):
    per_core, consts, meta, (core_of, local_of) = _prepare(cfg, **inputs)
    ncobj = _get_compiled(cfg, meta)
    in_maps = []
    for c in range(cfg.NC):
        pc = per_core[c]
        im = {"x4a": pc["x4a"], "x4b": pc["x4b"], "v1": pc["v1"],
              "s8": pc["s8"], "idx": pc["idx"]}
        im.update({k: np.asarray(v) for k, v in consts.items()})
        in_maps.append(im)
    res = run_bass_kernel_spmd(ncobj, in_maps, list(range(cfg.NC)),
                               trace=trace)
    g = np.empty(cfg.P, np.float32)
    for c in range(cfg.NC):
        go = np.asarray(res.results[c]["g"]).reshape(-1)
        mine = core_of == c
        g[mine] = go[local_of[mine]]
    return g.reshape(cfg.P, 1), res


def kernel(**inputs):
    cfg = Cfg(P=50000, E=800000)
    g, _ = run(cfg, inputs)
    return g


# revision 16
# speedup vs baseline: 1.7227x; 1.0021x over previous
"""BetaGNN message-passing kernel for 8 Trainium2 NeuronCores.

Strategy (dest-row sharding, 6250 nodes/core):
  - Host relabels nodes: sorted by in-degree, dealt round-robin to cores so
    every core's tile t has near-identical max-degree -> uniform chunk counts.
  - Hop 1 (AH = A @ relu(x @ W_in^T + b)): no gather. Host pre-gathers the
    3-wide input features per edge (plus a ones column); the PE recomputes h
    per edge-slot, TWO chunks per matmul (K=8 block-diagonal W_in, N=512).
    Edge values (x16) fold into the relu via per-partition scale; fp8
    messages accumulate FOUR chunks per DoubleRow identity matmul into a
    split [128,512] accumulator whose halves are summed in the epilogue.
  - Local AH rows (x16, fp8) are AllGathered in THREE slices, each fired as
    soon as its tiles finish so collectives overlap hop-1 compute and the
    early hop-2 gathers. Each slice lands in a compact table so gather
    indices stay int16.
  - Hop 2 (A2H = A @ AH): edges are bucketed by source slice and packed
    128/chunk with a general scatter matrix S (fp8, x16) routing
    slot -> dest row. Rows are dma_gathered (256B fp8) on 4 SWDGE queues;
    pairs of chunks accumulate with one DoubleRow matmul. Buckets are
    processed in separate passes (bf16 partials staged in SBUF) so a
    not-yet-ready collective never head-of-line blocks the gather queue;
    the next collective's dispatch is emitted in the middle of the previous
    bucket's gather stream.
  - Dense tail in transposed layout: AH/A2H tiles transpose via fp8 matmuls
    against scaled identities into [128, 2, NPAD] fp8 residents;
    h2^T = relu(W1 AH^T + W2 A2H^T) (DoubleRow over the two hid halves) and
    g = softplus(W_out h2^T + b_out), one 512-col block at a time,
    interleaved into the last hop-2 pass. All fp8 scale factors are powers
    of two (exact).
"""

import sys

for _p in ("/opt/trn_rl_repo", "/root/.axon_site/_ro/trn_rl_repo"):
    if _p not in sys.path:
        sys.path.insert(0, _p)

import numpy as np
import ml_dtypes

import concourse.bacc as bacc
import concourse.bass as bass
import concourse.mybir as mybir
from concourse import tile
from concourse.bass_utils import run_bass_kernel_spmd

F32 = mybir.dt.float32
F32R = mybir.dt.float32r
BF16 = mybir.dt.bfloat16
FP8 = mybir.dt.float8e4
I16 = mybir.dt.int16
AF = mybir.ActivationFunctionType
DR = mybir.MatmulPerfMode.DoubleRow
NPFP8 = ml_dtypes.float8_e4m3fn

MAX_CALL_CHUNKS = 12      # <=12 chunks (1536 idxs) per dma_gather call
NQUEUES = 4               # SWDGE queues for gather concurrency
COLL_FRAC = 0.6           # emit next collective after this fraction of calls


class Cfg:
    def __init__(self, P, E, nc=8, hid=256):
        assert P % (nc * 2) == 0
        self.P, self.E, self.NC, self.HID = P, E, nc, hid
        self.NPC = P // nc                    # nodes per core
        self.NT = (self.NPC + 127) // 128     # dest tiles per core
        self.NPAD = self.NT * 128
        if self.NT > 40:
            self.SPLITS = [17, 34]            # bucket boundaries (tiles)
        else:
            self.SPLITS = [max(1, self.NT // 2)]
        bounds = [0] + self.SPLITS + [self.NT]
        self.NB = len(bounds) - 1
        self.BROWS = []                       # locals per bucket
        for i in range(self.NB):
            lo = bounds[i] * 128
            hi = min(bounds[i + 1] * 128, self.NPC)
            self.BROWS.append(hi - lo)
        self.BT = bounds                      # tile bounds per bucket
        self.BLK = []
        off = 0
        while off < self.NPAD:
            w = min(512, self.NPAD - off)
            self.BLK.append((off, w))
            off += w


def _plan(cfg, deg):
    """Hop-1 plan: chunk count per tile = max in-degree in the tile."""
    P, NC, NT = cfg.P, cfg.NC, cfg.NT
    order = np.argsort(-deg, kind="stable")
    rank = np.empty(P, np.int64)
    rank[order] = np.arange(P)
    core_of = rank % NC
    local_of = rank // NC
    gid = core_of * cfg.NPC + local_of
    degs_sorted = deg[order]
    NCHUNK = []
    for t in range(NT):
        NCHUNK.append(max(1, int(degs_sorted[min(t * 128 * NC, P - 1)])))
    NCHUNK = np.array(NCHUNK, np.int64)
    tile_off = np.concatenate([[0], np.cumsum(NCHUNK)])
    return core_of, local_of, gid, NCHUNK, tile_off, int(tile_off[-1])


def _split_calls(nchunks):
    """Split a chunk count into gather calls <= MAX_CALL_CHUNKS, keeping
    every non-final call even so DoubleRow pairs never straddle calls."""
    out = []
    rem = nchunks
    while rem:
        g = min(MAX_CALL_CHUNKS, rem)
        if g < rem and g % 2:
            g -= 1
        out.append(g)
        rem -= g
    return out


def _pack_pairs(x4T, lo, hi):
    """Pack chunks [lo,hi) of x4T ([4, TC*128]) in h-pair layout: pair p ->
    partitions 32*(p%4)+(0..8), col block p//4. Odd tail chunk packs alone
    in the A-half of its pair slot."""
    n = hi - lo
    npr = (n + 1) // 2
    NQ = (npr + 3) // 4
    x4q = np.zeros((128, NQ * 128), np.float32)
    for p in range(npr):
        j, q = p % 4, p // 4
        kA = lo + 2 * p
        x4q[32 * j:32 * j + 4, q * 128:(q + 1) * 128] = \
            x4T[:, kA * 128:(kA + 1) * 128]
        if 2 * p + 1 < n:
            kB = kA + 1
            x4q[32 * j + 4:32 * j + 8, q * 128:(q + 1) * 128] = \
                x4T[:, kB * 128:(kB + 1) * 128]
    return x4q, NQ


def _prepare(cfg, beta, degree, A_rows, A_cols, A_vals,
             W_in, b_in, W_mp1, W_mp2, W_out, b_out):
    P, E, NC, NPC, NT = cfg.P, cfg.E, cfg.NC, cfg.NPC, cfg.NT
    NB = cfg.NB
    deg = np.bincount(A_rows, minlength=P).astype(np.int64)
    core_of, local_of, gid, NCHUNK, tile_off, TC = _plan(cfg, deg)

    # ---- hop-1 edge slots (slot column == dest column) ----
    d_gid = gid[A_rows.astype(np.int64)]
    oe = np.argsort(d_gid, kind="stable")
    sd = d_gid[oe]
    first = np.r_[True, sd[1:] != sd[:-1]]
    cumstart = np.maximum.accumulate(np.where(first, np.arange(E), 0))
    chunk = np.arange(E) - cumstart
    e_core = sd // NPC
    e_local = sd % NPC
    e_col = e_local % 128
    e_k = tile_off[e_local // 128] + chunk
    e_slot = e_k * 128 + e_col
    src1 = A_cols.astype(np.int64)[oe]
    vals1 = A_vals[oe].astype(np.float32)

    x4_all = np.stack([beta[:, 0], beta[:, 0] ** 2, degree[:, 0],
                       np.ones(P, np.float32)], axis=0).astype(np.float32)

    # ---- hop-2 edge plan: sort by (core, tile, bucket) ----
    s_gid = gid[A_cols.astype(np.int64)]
    c2_core = d_gid // NPC
    c2_loc = d_gid % NPC
    c2_tile = c2_loc // 128
    c2_dcol = c2_loc % 128
    s_loc = s_gid % NPC
    s_core = s_gid // NPC
    blo = np.array([cfg.BT[i] * 128 for i in range(NB)], np.int64)
    c2_b = np.searchsorted(blo, s_loc, side="right") - 1
    brows = np.array(cfg.BROWS, np.int64)
    c2_tidx = s_core * brows[c2_b] + (s_loc - blo[c2_b])
    o2 = np.lexsort((c2_b, c2_tile, c2_core))
    g_core = c2_core[o2]
    g_tile = c2_tile[o2]
    g_b = c2_b[o2]
    g_dcol = c2_dcol[o2]
    g_tidx = c2_tidx[o2]
    g_val = A_vals[o2].astype(np.float32)
    key = (g_core * NT + g_tile) * NB + g_b
    kfirst = np.r_[True, key[1:] != key[:-1]]
    kcum = np.maximum.accumulate(np.where(kfirst, np.arange(E), 0))
    g_pos = np.arange(E) - kcum

    # shared SPMD structure (max over cores, min 1 chunk per (t,b))
    cnt_all = np.zeros((NC, NT, NB), np.int64)
    np.add.at(cnt_all, (g_core, g_tile, g_b), 1)
    nch = np.maximum(1, -(-cnt_all.max(axis=0) // 128))   # [NT, NB]
    flat = nch.reshape(-1)
    cbase = np.concatenate([[0], np.cumsum(flat)]).astype(np.int64)
    TOT = int(cbase[-1])
    # calls grouped bucket-major (pass order)
    calls = []          # (tile, bucket, chunk_base, g)
    for b in range(NB):
        for t in range(NT):
            base = int(cbase[t * NB + b])
            for g in _split_calls(int(nch[t, b])):
                calls.append((t, b, base, g))
                base += g
    NIC = sum(g * 128 // 16 for (_, _, _, g) in calls)

    part_bounds = [int(tile_off[bt]) for bt in cfg.BT]    # chunk bounds

    per_core = []
    for c in range(NC):
        # ---- hop 1 arrays ----
        m1 = e_core == c
        sl1 = e_slot[m1]
        x4T = np.zeros((4, TC * 128), np.float32)
        x4T[:, sl1] = x4_all[:, src1[m1]]
        v1 = np.zeros((128, TC), np.float32)
        v1[e_col[m1], e_k[m1]] = 16.0 * vals1[m1]
        xparts = []
        for i in range(NB):
            x4q, NQ = _pack_pairs(x4T, part_bounds[i], part_bounds[i + 1])
            xparts.append(x4q)

        # ---- hop 2 arrays ----
        m2 = g_core == c
        e_key = (g_tile[m2] * NB + g_b[m2])
        e_chunk = cbase[e_key] + g_pos[m2] // 128
        e_p = g_pos[m2] % 128
        S8 = np.zeros((128, TOT * 128), np.float32)
        S8[e_p, e_chunk * 128 + g_dcol[m2]] = 16.0 * g_val[m2]
        S8 = S8.astype(NPFP8)
        slot_idx = np.zeros(TOT * 128, np.int64)
        slot_idx[e_chunk * 128 + e_p] = g_tidx[m2]

        idxh = np.zeros((128, NIC), np.int16)
        col0 = 0
        for (t, b, base, g) in calls:
            ni = g * 128
            blockv = slot_idx[base * 128:base * 128 + ni].astype(np.int16)
            blockv = blockv.reshape(ni // 16, 16).T
            for q in range(8):
                idxh[16 * q:16 * (q + 1), col0:col0 + ni // 16] = blockv
            col0 += ni // 16
        pc = dict(v1=v1, s8=S8, idx=idxh)
        for i in range(NB):
            pc[f"x4_{i}"] = xparts[i]
        per_core.append(pc)

    # ---- constants (power-of-two scaled for fp8) ----
    wiT = np.concatenate([W_in.T.astype(np.float32),
                          b_in[None, :].astype(np.float32)], axis=0)
    HID = cfg.HID
    wiT2 = np.zeros((128, 2 * HID), np.float32)
    for j in range(4):
        wiT2[32 * j:32 * j + 4, 0:HID] = wiT
        wiT2[32 * j + 4:32 * j + 8, HID:2 * HID] = wiT

    def pack_w(W, scale):
        w = (scale * W.T.astype(np.float32)).reshape(2, 128, W.shape[0])
        return np.ascontiguousarray(w.transpose(1, 0, 2)).astype(NPFP8)

    idn2 = np.zeros((128, 2, 128), np.float32)
    idn2[np.arange(128), 0, np.arange(128)] = 1.0
    idn2[np.arange(128), 1, np.arange(128)] = 1.0

    consts = dict(
        wit2=wiT2,
        w1s=pack_w(W_mp1, 32.0),
        w2s=pack_w(W_mp2, 16.0),
        wos=(16.0 * W_out.reshape(2, 128).T.reshape(128, 2, 1)
             .astype(np.float32)).astype(NPFP8),
        bout=np.full((128, 1), float(np.asarray(b_out).reshape(-1)[0]),
                     np.float32),
        idn2=idn2.astype(NPFP8),
        sidn8=(np.eye(128, dtype=np.float32) * 0.125).astype(NPFP8),
        sidn4=(np.eye(128, dtype=np.float32) * 0.25).astype(NPFP8),
    )
    meta = dict(NCHUNK=tuple(int(x) for x in NCHUNK), TC=TC,
                nch=tuple(int(x) for r in nch for x in r),
                calls=tuple(calls), TOT=TOT, NIC=NIC,
                NQs=tuple((((part_bounds[i + 1] - part_bounds[i]) + 1) // 2
                           + 3) // 4 for i in range(NB)))
    return per_core, consts, meta, (core_of, local_of)


def _build(cfg, meta):
    NT, NPC, NPAD, HID, NC, P, NB = (cfg.NT, cfg.NPC, cfg.NPAD, cfg.HID,
                                     cfg.NC, cfg.P, cfg.NB)
    NCHUNK = meta["NCHUNK"]
    TC, NIC, TOT = meta["TC"], meta["NIC"], meta["TOT"]
    calls = meta["calls"]
    nch = np.array(meta["nch"], np.int64).reshape(NT, NB)
    tile_off = np.concatenate([[0], np.cumsum(NCHUNK)])
    NBLK = len(cfg.BLK)
    NQs = meta["NQs"]
    part_bounds = [int(tile_off[bt]) for bt in cfg.BT]

    nc = bacc.Bacc("TRN2", target_bir_lowering=False, debug=False,
                   num_swdge_queues=NQUEUES)
    x4_d = [nc.dram_tensor(f"x4_{i}", [128, max(NQs[i], 1) * 128], F32R,
                           kind="ExternalInput") for i in range(NB)]
    v1_d = nc.dram_tensor("v1", [128, TC], F32, kind="ExternalInput")
    s8_d = nc.dram_tensor("s8", [128, TOT * 128], FP8, kind="ExternalInput")
    idx_d = nc.dram_tensor("idx", [128, NIC], I16, kind="ExternalInput")
    wiT2_d = nc.dram_tensor("wit2", [128, 2 * HID], F32R,
                            kind="ExternalInput")
    w1s_d = nc.dram_tensor("w1s", [128, 2 * HID], FP8, kind="ExternalInput")
    w2s_d = nc.dram_tensor("w2s", [128, 2 * HID], FP8, kind="ExternalInput")
    wos_d = nc.dram_tensor("wos", [128, 2], FP8, kind="ExternalInput")
    bout_d = nc.dram_tensor("bout", [128, 1], F32, kind="ExternalInput")
    idn2_d = nc.dram_tensor("idn2", [128, 2 * 128], FP8, kind="ExternalInput")
    sidn8_d = nc.dram_tensor("sidn8", [128, 128], FP8, kind="ExternalInput")
    sidn4_d = nc.dram_tensor("sidn4", [128, 128], FP8, kind="ExternalInput")
    g_d = nc.dram_tensor("g", [1, NBLK * 512], F32, kind="ExternalOutput")

    bounce = [nc.dram_tensor(f"bounce{i}", [cfg.BROWS[i], HID], FP8)
              for i in range(NB)]
    table = [nc.dram_tensor(f"table{i}", [NC * cfg.BROWS[i], HID], FP8,
                            addr_space="Shared") for i in range(NB)]

    with tile.TileContext(nc) as tc:
        with (
            tc.tile_pool(name="const", bufs=1) as constp,
            tc.tile_pool(name="xs", bufs=3) as xsp,
            tc.tile_pool(name="msgs", bufs=6) as msgp,
            tc.tile_pool(name="sd", bufs=8) as sdp,
            tc.tile_pool(name="stage", bufs=3) as stagep,
            tc.tile_pool(name="resid", bufs=1) as residp,
            tc.tile_pool(name="pair", bufs=24) as pairp,
            tc.tile_pool(name="ph", bufs=2, space="PSUM") as php,
            tc.tile_pool(name="pz", bufs=2, space="PSUM") as pzp,
            tc.tile_pool(name="pt", bufs=2, space="PSUM") as ptp,
        ):
            wiT2 = constp.tile([128, 2 * HID], F32R, tag="wiT2", name="wiT2")
            nc.sync.dma_start(wiT2[:], wiT2_d[:])
            w1s = constp.tile([128, 2, HID], FP8, tag="w1s", name="w1s")
            nc.sync.dma_start(w1s[:], w1s_d[:])
            w2s = constp.tile([128, 2, HID], FP8, tag="w2s", name="w2s")
            nc.sync.dma_start(w2s[:], w2s_d[:])
            wos = constp.tile([128, 2, 1], FP8, tag="wos", name="wos")
            nc.sync.dma_start(wos[:], wos_d[:])
            bout = constp.tile([128, 1], F32, tag="bout", name="bout")
            nc.sync.dma_start(bout[:], bout_d[:])
            idn2 = constp.tile([128, 2, 128], FP8, tag="idn2", name="idn2")
            nc.sync.dma_start(idn2[:], idn2_d[:])
            sidn8 = constp.tile([128, 128], FP8, tag="sidn8", name="sidn8")
            nc.sync.dma_start(sidn8[:], sidn8_d[:])
            sidn4 = constp.tile([128, 128], FP8, tag="sidn4", name="sidn4")
            nc.sync.dma_start(sidn4[:], sidn4_d[:])
            v1 = constp.tile([128, TC], F32, tag="v1", name="v1")
            nc.sync.dma_start(v1[:], v1_d[:])
            idx = constp.tile([128, NIC], I16, tag="idx", name="idx")
            nc.sync.dma_start(idx[:], idx_d[:])

            ahT = residp.tile([128, 2, NPAD], FP8, tag="ahT", name="ahT")
            a2T = residp.tile([128, 2, NPAD], FP8, tag="a2T", name="a2T")
            partial = residp.tile([128, NT, HID], BF16, tag="part",
                                  name="part")

            # ---- phase A: hop 1 ------------------------------------------
            def epilogue_a(t, pz, used_right, part_i):
                ahb = stagep.tile([128, HID], FP8, tag="ahb", name="ahb")
                if used_right:
                    rh = stagep.tile([128, HID], BF16, tag="rh", name="rh")
                    nc.scalar.activation(rh[:], pz[:, HID:2 * HID], AF.Copy)
                    nc.vector.tensor_tensor(
                        ahb[:], pz[:, :HID], rh[:],
                        op=mybir.AluOpType.add)
                else:
                    nc.scalar.activation(ahb[:], pz[:, :HID], AF.Copy)
                r0 = t * 128 - cfg.BT[part_i] * 128
                rows = min(128, NPC - t * 128)
                nc.sync.dma_start(bounce[part_i][r0:r0 + rows, :],
                                  ahb[:rows, :])
                for mh in (0, 1):
                    pt = ptp.tile([128, 512], F32, tag="pt", name="pt")
                    nc.tensor.matmul(
                        pt[:, :128], lhsT=ahb[:, mh * 128:(mh + 1) * 128],
                        rhs=sidn8[:], start=True, stop=True,
                        skip_group_check=True)
                    nc.vector.tensor_copy(
                        ahT[:, mh, t * 128:(t + 1) * 128], pt[:, :128])

            def phase_a(part_i):
                lo, hi = part_bounds[part_i], part_bounds[part_i + 1]
                xd = x4_d[part_i]
                NQp = NQs[part_i]
                t = int(np.searchsorted(tile_off, lo, side="right")) - 1
                pz = None
                mq = None          # quad message tile [128, 2, 512]
                mq2 = None         # leftover pair tile [128, 2, 256]
                xs = None
                for p in range((hi - lo + 1) // 2):
                    if p % 4 == 0:
                        xs = xsp.tile([128, 128], F32R, tag="xs", name="xs")
                        q = p // 4
                        nc.sync.dma_start(xs[:],
                                          xd[:, q * 128:(q + 1) * 128])
                    j = p % 4
                    kA = lo + 2 * p
                    single = kA + 1 >= hi
                    ph = php.tile([128, 512], F32, tag="ph", name="ph",
                                  bufs=4)
                    nc.tensor.matmul(
                        ph[:, :2 * HID],
                        lhsT=xs[32 * j:32 * j + 8, :],
                        rhs=wiT2[32 * j:32 * j + 8, :],
                        start=True, stop=True, skip_group_check=True,
                        tile_position=(32 * j, 0))
                    for k in (kA,) if single else (kA, kA + 1):
                        if k == int(tile_off[t]):
                            pz = pzp.tile([128, 512], F32, tag="acc",
                                          name="acc")
                        nchk = int(NCHUNK[t])
                        q_in = k - int(tile_off[t])
                        nq = nchk // 4
                        tstart = q_in == 0
                        tlast = q_in == nchk - 1
                        ph_half = ph[:, (k - kA) * HID:(k - kA + 1) * HID]
                        # destination quarter for this chunk's message
                        if q_in < 4 * nq:
                            qq = q_in % 4
                            if qq == 0:
                                mq = msgp.tile([128, 2, 2 * HID], FP8,
                                               tag="mq", name="mq")
                            dst = mq[:, qq % 2, (qq // 2) * HID:
                                     (qq // 2 + 1) * HID]
                        else:
                            rr = q_in - 4 * nq
                            if rr == 0:
                                mq2 = msgp.tile([128, 2, HID], FP8,
                                                tag="mq2", name="mq2")
                            dst = mq2[:, rr % 2, :] if rr < 2 \
                                else mq2[:, 0, :]
                            if rr == 2:
                                mq2 = msgp.tile([128, 2, HID], FP8,
                                                tag="mq2", name="mq2")
                                dst = mq2[:, 0, :]
                        if k % 2 == 0:
                            nc.scalar.activation(dst, ph_half, AF.Relu,
                                                 scale=v1[:, k:k + 1])
                        else:
                            nc.vector.tensor_scalar(
                                dst, ph_half, v1[:, k:k + 1], 0.0,
                                op0=mybir.AluOpType.mult,
                                op1=mybir.AluOpType.max)
                        # emit accumulation matmuls
                        if q_in < 4 * nq and q_in % 4 == 3:
                            nc.tensor.matmul(
                                pz[:, :2 * HID], lhsT=idn2[:], rhs=mq[:],
                                perf_mode=DR, start=(q_in == 3),
                                stop=tlast, skip_group_check=True)
                        elif q_in >= 4 * nq:
                            rr = q_in - 4 * nq
                            rem = nchk - 4 * nq
                            if rr == 1 and rem >= 2:
                                nc.tensor.matmul(
                                    pz[:, :HID], lhsT=idn2[:], rhs=mq2[:],
                                    perf_mode=DR, start=(nq == 0 and rr == 1),
                                    stop=(q_in == nchk - 1),
                                    skip_group_check=True)
                            elif rr == 0 and rem == 1 or rr == 2:
                                nc.tensor.matmul(
                                    pz[:, :HID], lhsT=idn2[:, 0, :],
                                    rhs=mq2[:, 0, :],
                                    start=(nq == 0 and rr == 0),
                                    stop=tlast, skip_group_check=True)
                        if tlast:
                            epilogue_a(t, pz, nq > 0, part_i)
                            t += 1

            # ---- emit phase A parts + collectives + phase C passes -------
            for i in range(NB):
                phase_a(i)

            def collective(i):
                nc.gpsimd.collective_compute(
                    "AllGather", mybir.AluOpType.bypass,
                    replica_groups=[list(range(NC))],
                    ins=[bounce[i].ap().opt()],
                    outs=[table[i].ap().opt()],
                )

            collective(0)

            def dense_block(bidx):
                off, w = cfg.BLK[bidx]
                ht = stagep.tile([128, 2, 512], FP8, tag="h2t", name="h2t")
                for mh in (0, 1):
                    pd = pzp.tile([128, 512], F32, tag="acc", name="acc")
                    nc.tensor.matmul(
                        pd[:, :w], lhsT=w1s[:, :, mh * 128:(mh + 1) * 128],
                        rhs=ahT[:, :, off:off + w], perf_mode=DR,
                        start=True, stop=False, skip_group_check=True)
                    nc.tensor.matmul(
                        pd[:, :w], lhsT=w2s[:, :, mh * 128:(mh + 1) * 128],
                        rhs=a2T[:, :, off:off + w], perf_mode=DR,
                        start=False, stop=True, skip_group_check=True)
                    nc.scalar.activation(ht[:, mh, :w], pd[:, :w], AF.Relu,
                                         scale=0.015625)
                pg = ptp.tile([1, 512], F32, tag="pt", name="pg")
                for i in (0, 1):
                    nc.tensor.matmul(pg[:, :w], lhsT=wos[:, i, :],
                                     rhs=ht[:, i, :w],
                                     start=(i == 0), stop=(i == 1),
                                     skip_group_check=True)
                gb = stagep.tile([1, 512], F32, tag="gbuf", name="gb",
                                 bufs=4)
                nc.vector.tensor_copy(gb[0:1, :w], pg[:, :w])
                ge = stagep.tile([1, 512], F32, tag="gbuf", name="ge",
                                 bufs=4)
                nc.scalar.activation(ge[0:1, :w], gb[0:1, :w], AF.Exp,
                                     bias=bout[0:1, :], scale=0.0625)
                go = stagep.tile([1, 512], F32, tag="gbuf", name="go",
                                 bufs=4)
                nc.scalar.activation(go[0:1, :w], ge[0:1, :w], AF.Ln,
                                     bias=1.0)
                nc.sync.dma_start(g_d[0:1, off:off + w], go[0:1, :w])

            # phase C: one pass per bucket
            ci = 0
            col0 = 0
            qrr = 0
            for b in range(NB):
                bcalls = [cl for cl in calls if cl[1] == b]
                ncoll = max(1, int(COLL_FRAC * len(bcalls)))
                nc_done = 0
                for t in range(NT):
                    ncht = int(nch[t, b])
                    pz = pzp.tile([128, 512], F32, tag="acc", name="acc")
                    done = 0
                    while done < ncht:
                        (tt, bb, base, g) = calls[ci]
                        assert tt == t and bb == b
                        ni = g * 128
                        pr = pairp.tile([128, MAX_CALL_CHUNKS, HID], FP8,
                                        tag="pair", name="pair")
                        nc.gpsimd.dma_gather(
                            pr[:, :g, :], table[b].ap(),
                            idx[:, col0:col0 + ni // 16],
                            ni, ni, HID, single_packet=False,
                            queue_num=qrr)
                        qrr = (qrr + 1) % NQUEUES
                        sd = sdp.tile([128, MAX_CALL_CHUNKS, 128], FP8,
                                      tag="sdl", name="sdl")
                        nc.sync.dma_start(
                            sd[:, :g, :],
                            s8_d[:, base * 128:(base + g) * 128])
                        for cc in range(0, g - 1, 2):
                            nc.tensor.matmul(
                                pz[:, :HID], lhsT=sd[:, cc:cc + 2, :],
                                rhs=pr[:, cc:cc + 2, :], perf_mode=DR,
                                start=(done + cc == 0),
                                stop=(done + cc + 2 == ncht),
                                skip_group_check=True)
                        if g % 2:
                            nc.tensor.matmul(
                                pz[:, :HID], lhsT=sd[:, g - 1, :],
                                rhs=pr[:, g - 1, :],
                                start=(done + g - 1 == 0),
                                stop=(done + g == ncht),
                                skip_group_check=True)
                        done += g
                        col0 += ni // 16
                        ci += 1
                        nc_done += 1
                        if b + 1 < NB and nc_done == ncoll:
                            collective(b + 1)
                    # combine into partial / final epilogue
                    if b + 1 < NB:
                        if b == 0:
                            nc.vector.tensor_scalar(
                                partial[:, t, :], pz[:, :HID], 0.0625, 0.0,
                                op0=mybir.AluOpType.mult,
                                op1=mybir.AluOpType.bypass)
                        else:
                            t1 = stagep.tile([128, HID], BF16, tag="t1",
                                             name="t1")
                            nc.vector.tensor_scalar(
                                t1[:], pz[:, :HID], 0.0625, 0.0,
                                op0=mybir.AluOpType.mult,
                                op1=mybir.AluOpType.bypass)
                            nc.vector.tensor_tensor(
                                partial[:, t, :], partial[:, t, :], t1[:],
                                op=mybir.AluOpType.add)
                    else:
                        t1 = stagep.tile([128, HID], BF16, tag="t1",
                                         name="t1")
                        nc.vector.tensor_scalar(
                            t1[:], pz[:, :HID], 0.0625, 0.0,
                            op0=mybir.AluOpType.mult,
                            op1=mybir.AluOpType.bypass)
                        a2b = stagep.tile([128, HID], FP8, tag="a2b",
                                          name="a2b")
                        nc.vector.tensor_tensor(
                            a2b[:], partial[:, t, :], t1[:],
                            op=mybir.AluOpType.add)
                        for mh in (0, 1):
                            pt = ptp.tile([128, 512], F32, tag="pt",
                                          name="pt")
                            nc.tensor.matmul(
                                pt[:, :128],
                                lhsT=a2b[:, mh * 128:(mh + 1) * 128],
                                rhs=sidn4[:], start=True, stop=True,
                                skip_group_check=True)
                            nc.vector.tensor_copy(
                                a2T[:, mh, t * 128:(t + 1) * 128],
                                pt[:, :128])
                        if t % 4 == 3:
                            dense_block(t // 4)
            for bidx in range(NT // 4, NBLK):
                dense_block(bidx)

    nc.compile()
    return nc


_COMPILED = {}


def _get_compiled(cfg, meta):
    key = (cfg.P, cfg.E, meta["NCHUNK"], meta["nch"], meta["calls"])
    if key not in _COMPILED:
        _COMPILED[key] = _build(cfg, meta)
    return _COMPILED[key]


def run(cfg, inputs, trace=False):
    per_core, consts, meta, (core_of, local_of) = _prepare(cfg, **inputs)
    ncobj = _get_compiled(cfg, meta)
    in_maps = []
    for c in range(cfg.NC):
        im = dict(per_core[c])
        im.update({k: np.asarray(v) for k, v in consts.items()})
        in_maps.append(im)
    res = run_bass_kernel_spmd(ncobj, in_maps, list(range(cfg.NC)),
                               trace=trace)
    g = np.empty(cfg.P, np.float32)
    for c in range(cfg.NC):
        go = np.asarray(res.results[c]["g"]).reshape(-1)
        mine = core_of == c
        g[mine] = go[local_of[mine]]
    return g.reshape(cfg.P, 1), res


def kernel(**inputs):
    cfg = Cfg(P=50000, E=800000)
    g, _ = run(cfg, inputs)
    return g
